# revision 20
# baseline (speedup 1.0000x reference)
"""Trainium2 Bass kernel for nn_CrossAttention (spiking cross-attention).

Math (per (t,b) pair, derived from the reference):
  qt   = query + query_pos                      (NQ,C) == (C,NQ) flat relabel
  qh   = 1{qt >= 2.5}                           binary  (quant4(spike_norm4))
  kin  = key + key_pos, viewed as (C,NK) flat relabel
  khq  = round(clamp(kin, 0, 4))                integers 0..4
  Kq   = max(0, round(Wk'·khq + cbk))           Wk' folds 0.25·diag(k_s)·k_w
  Vq   = max(0, round(Wv'·khq + cbv))
  S_h  = Kq_h^T @ Vq_h  (32x32 per head, contract NK)   [exact ints in fp32]
  Xint = S_h^T @ Qq_h   where Qq = max(0, round(Wq'·qh + cbq))
  xq   = clamp(round(Xint * 0.1/64), 0, 4)
  out  = Wp'·xq + cbp  (+ qt)                   Wp' folds 0.25

Hardware tricks (all verified exact on device):
  - elementwise adds (query+query_pos, key+key_pos) via accumulating
    software-DGE DMAs (accum_op=add) - zero vector-engine cost
  - round-to-nearest-even + clamp-below-0 via fp32->uint8 conversion
    (DVE/ACT output converter rounds RNE and saturates)
  - uint8->fp16 via casting DMAs / ACT copies
  - upper clamp dropped where |value| provably < 4.5 (conv outputs)
  - K and V convs share one rhs [WkT | WvT] (fewer LDWEIGHTS, fused PSUM)
Sharding: T*B = 8 pairs -> 8 cores, no collectives.
"""

import numpy as np

import concourse.bass as bass
import concourse.mybir as mybir
import concourse.tile as tile
from concourse import bacc
from concourse.bass_utils import run_bass_kernel_spmd

T, B, NQ, NK, C, H = 4, 2, 256, 4096, 256, 8
NCORES = T * B
MAGIC = float(np.float32(12582912.0))  # 1.5 * 2**23: fp32 RNE rounding shift
SCALE_X = float(np.float32(np.float32(0.1) / np.float32(64.0)))
F32 = mybir.dt.float32
F16 = mybir.dt.float16
U8 = mybir.dt.uint8

_CACHE = {}


def _build_nc():
    nc = bacc.Bacc(
        "TRN2",
        target_bir_lowering=False,
        debug=False,
        enable_asserts=False,
        num_devices=NCORES,
    )
    d_query = nc.dram_tensor("query", [C, NQ], F32, kind="ExternalInput")
    d_qpos = nc.dram_tensor("query_pos", [C, NQ], F32, kind="ExternalInput")
    d_key = nc.dram_tensor("key", [C, NK], F32, kind="ExternalInput")
    d_kpos = nc.dram_tensor("key_pos", [C, NK], F32, kind="ExternalInput")
    d_wq = nc.dram_tensor("wq_t", [C, C], F16, kind="ExternalInput")
    d_wkv = nc.dram_tensor("wkv_t", [C, 512], F16, kind="ExternalInput")
    d_wp = nc.dram_tensor("wp_t", [C, C], F16, kind="ExternalInput")
    d_cbkv = nc.dram_tensor("cbkv", [1, 512], F32, kind="ExternalInput")
    d_cq = nc.dram_tensor("cq", [C, 1], F32, kind="ExternalInput")
    d_cp = nc.dram_tensor("cp", [C, 1], F32, kind="ExternalInput")
    d_out = nc.dram_tensor("out", [C, NQ], F32, kind="ExternalOutput")

    AL = mybir.AluOpType
    AF = mybir.ActivationFunctionType
    NG = 8  # groups of 512 key positions

    with tile.TileContext(nc) as tc:
        with (
            tc.tile_pool(name="const", bufs=1) as const,
            tc.tile_pool(name="io", bufs=5) as io,
            tc.tile_pool(name="work", bufs=5) as work,
            tc.tile_pool(name="ev", bufs=5) as ev,
            tc.tile_pool(name="small", bufs=2) as small,
            tc.tile_pool(name="pkv", bufs=2, space="PSUM") as pkv,
            tc.tile_pool(name="ps", bufs=1, space="PSUM") as ps,
            tc.tile_pool(name="psq", bufs=2, space="PSUM") as psq,
        ):
            # ---- constants / weights ----
            wq_sb = const.tile([128, 2, C], F16, tag="wq")
            wp_sb = const.tile([128, 2, C], F16, tag="wp")
            wkv_sb = const.tile([128, 2, 512], F16, tag="wkv")
            for ct in range(2):
                nc.sync.dma_start(out=wq_sb[:, ct, :], in_=d_wq[128 * ct : 128 * ct + 128, :])
                nc.sync.dma_start(out=wp_sb[:, ct, :], in_=d_wp[128 * ct : 128 * ct + 128, :])
                nc.sync.dma_start(out=wkv_sb[:, ct, :], in_=d_wkv[128 * ct : 128 * ct + 128, :])
            cbkv_sb = const.tile([128, 2, 512], F32, tag="cbkv")
            a = d_cbkv[:]
            bcast = bass.AP(tensor=a.tensor, offset=a.offset, ap=[[0, 128], [0, 2], [1, 512]])
            nc.sync.dma_start(out=cbkv_sb[:], in_=bcast)
            cq_sb = const.tile([128, 2, 1], F32, tag="cq")
            cp_sb = const.tile([128, 2, 1], F32, tag="cp")
            for t_, dram in ((cq_sb, d_cq), (cp_sb, d_cp)):
                for ct in range(2):
                    nc.sync.dma_start(out=t_[:, ct, :], in_=dram[128 * ct : 128 * ct + 128, :])

            # ---- q path: qt (add via accum-DMA), qh, Q conv, Qq ----
            qt_sb = const.tile([128, 2, NQ], F32, tag="qt")
            qh_sb = const.tile([128, 2, NQ], F16, tag="qh")
            for ct in range(2):
                nc.sync.dma_start(out=qt_sb[:, ct, :], in_=d_query[128 * ct : 128 * ct + 128, :])
            for ct in range(2):
                nc.gpsimd.dma_start(out=qt_sb[:, ct, :], in_=d_qpos[128 * ct : 128 * ct + 128, :], accum_op=AL.add)
            nc.vector.tensor_scalar(qh_sb[:], qt_sb[:], 2.5, None, AL.is_ge)

            qq_sb = const.tile([128, 2, NQ], F32, tag="qq")
            for o in range(2):
                p_q = psq.tile([128, NQ], F32, tag="psq")
                for ct in range(2):
                    nc.tensor.matmul(
                        p_q[:],
                        wq_sb[:, ct, 128 * o : 128 * o + 128],
                        qh_sb[:, ct, :],
                        start=(ct == 0),
                        stop=(ct == 1),
                    )
                yq = small.tile([128, NQ], F32, tag="yq")
                nc.vector.tensor_scalar(yq[:], p_q[:], cq_sb[:, o, :], MAGIC, AL.add, AL.add)
                nc.vector.tensor_scalar(qq_sb[:, o, :], yq[:], MAGIC, 0.0, AL.subtract, AL.max)

            # ---- main loop over key groups ----
            p_sA = ps.tile([128, 2, 128], F32, tag="s")  # [:,0,:]=heads 0-3, [:,1,:]=heads 4-7
            for g in range(NG):
                n0 = 512 * g
                kin = io.tile([128, 2, 512], F32, tag="kin")
                kq_eng = nc.sync if g % 2 == 0 else nc.scalar
                kk = d_key[:]
                kk3 = bass.AP(
                    tensor=kk.tensor,
                    offset=kk.offset + n0,
                    ap=[[NK, 128], [128 * NK, 2], [1, 512]],
                )
                kq_eng.dma_start(out=kin[:], in_=kk3)
                kp = d_kpos[:]
                kp3 = bass.AP(
                    tensor=kp.tensor,
                    offset=kp.offset + n0,
                    ap=[[NK, 128], [128 * NK, 2], [1, 512]],
                )
                nc.gpsimd.dma_start(out=kin[:], in_=kp3, accum_op=AL.add)
                rr = work.tile([128, 2, 512], F32, tag="rr")
                nc.vector.tensor_scalar(rr[:], kin[:], MAGIC, MAGIC, AL.add, AL.max)
                khq = work.tile([128, 2, 512], F16, tag="khq")
                nc.vector.tensor_scalar(khq[:], rr[:], MAGIC + 4.0, MAGIC, AL.min, AL.subtract)

                for p in range(2):
                    # psum tile: [:, s, 0:256] = Kq conv, [:, s, 256:512] = Vq conv
                    p_kv = pkv.tile([128, 2, 512], F32, tag="pkv")
                    for s in range(2):
                        lo = 256 * p + 128 * s
                        for ct in range(2):
                            nc.tensor.matmul(
                                p_kv[:, s, :],
                                khq[:, ct, lo : lo + 128],
                                wkv_sb[:, ct, :],
                                start=(ct == 0),
                                stop=(ct == 1),
                            )
                    evu = ev.tile([128, 2, 512], U8, tag="evu")
                    nc.vector.tensor_tensor(evu[:], p_kv[:], cbkv_sb[:], AL.add)
                    evf = ev.tile([128, 2, 512], F16, tag="evf")
                    nc.scalar.activation(evf[:], evu[:], AF.Copy, bias=0.0)
                    first = g == 0 and p == 0
                    last = g == NG - 1 and p == 1
                    for s in range(2):
                        for hf in range(2):
                            nc.tensor.matmul(
                                p_sA[:, hf, :],
                                evf[:, s, 128 * hf : 128 * hf + 128],
                                evf[:, s, 256 + 128 * hf : 256 + 128 * hf + 128],
                                start=(first and s == 0),
                                stop=(last and s == 1),
                            )

            # ---- S -> block-diagonal SBUF copies ----
            s_sb = const.tile([128, 2, 128], F32, tag="ssb")
            nc.vector.memset(s_sb[:], 0.0)
            for hf in range(2):
                for j in range(4):
                    nc.scalar.activation(
                        s_sb[32 * j : 32 * j + 32, hf, 32 * j : 32 * j + 32],
                        p_sA[32 * j : 32 * j + 32, hf, 32 * j : 32 * j + 32],
                        AF.Copy,
                        bias=0.0,
                    )

            # ---- X = S^T @ Qq (block-diag masked), quantize ----
            xq_sb = const.tile([128, 2, NQ], F16, tag="xq")
            for hf in range(2):
                p_x = psq.tile([128, NQ], F32, tag="psq")
                nc.tensor.matmul(p_x[:], s_sb[:, hf, :], qq_sb[:, hf, :], start=True, stop=True)
                yxu = small.tile([128, NQ], U8, tag="yxu")
                nc.vector.tensor_scalar(yxu[:], p_x[:], SCALE_X, 4.49, AL.mult, AL.min)
                nc.scalar.activation(xq_sb[:, hf, :], yxu[:], AF.Copy, bias=0.0)

            # ---- P conv + bias + qt, store ----
            for o in range(2):
                p_p = psq.tile([128, NQ], F32, tag="psq")
                for ct in range(2):
                    nc.tensor.matmul(
                        p_p[:],
                        wp_sb[:, ct, 128 * o : 128 * o + 128],
                        xq_sb[:, ct, :],
                        start=(ct == 0),
                        stop=(ct == 1),
                    )
                yp = small.tile([128, NQ], F32, tag="yp")
                nc.scalar.activation(yp[:], p_p[:], AF.Identity, bias=cp_sb[:, o, :])
                osb = small.tile([128, NQ], F32, tag="osb")
                nc.vector.tensor_add(osb[:], yp[:], qt_sb[:, o, :])
                nc.sync.dma_start(out=d_out[128 * o : 128 * o + 128, :], in_=osb[:])

    nc.compile()
    return nc


def _host_fold(q_w, q_b, q_s, q_o, k_w, k_b, k_s, k_o, v_w, v_b, v_s, v_o, p_w, p_b, p_s, p_o):
    def fold(w, b, s, o, pre):
        wf = (pre * s[:, None] * w).T.astype(np.float16)  # (C_in, C_out)
        cb = (s * b + o).astype(np.float32)
        return np.ascontiguousarray(wf), cb

    wq, cbq = fold(q_w, q_b, q_s, q_o, 1.0)
    wk, cbk = fold(k_w, k_b, k_s, k_o, 0.25)
    wv, cbv = fold(v_w, v_b, v_s, v_o, 0.25)
    wp, cbp = fold(p_w, p_b, p_s, p_o, 0.25)
    return {
        "wq_t": wq,
        "wkv_t": np.ascontiguousarray(np.concatenate([wk, wv], axis=1)),
        "wp_t": wp,
        "cbkv": np.concatenate([cbk, cbv])[None, :].astype(np.float32),
        "cq": cbq[:, None].astype(np.float32),
        "cp": cbp[:, None].astype(np.float32),
    }


def kernel(query, key, value, query_pos, key_pos,
           q_w, q_b, q_s, q_o, k_w, k_b, k_s, k_o,
           v_w, v_b, v_s, v_o, p_w, p_b, p_s, p_o,
           _trace=False):
    del value  # the reference ignores it (vh = kh)
    if "nc" not in _CACHE:
        _CACHE["nc"] = _build_nc()
    nc = _CACHE["nc"]

    shared = _host_fold(q_w, q_b, q_s, q_o, k_w, k_b, k_s, k_o,
                        v_w, v_b, v_s, v_o, p_w, p_b, p_s, p_o)
    query = np.asarray(query, np.float32)
    query_pos = np.asarray(query_pos, np.float32)
    key = np.asarray(key, np.float32)
    key_pos = np.asarray(key_pos, np.float32)

    in_maps = []
    for cid in range(NCORES):
        t, b = cid // B, cid % B
        m = dict(shared)
        m["query"] = query[t, b].reshape(C, NQ)
        m["query_pos"] = query_pos[t, b].reshape(C, NQ)
        m["key"] = key[t, b].reshape(C, NK)
        m["key_pos"] = key_pos[t, b].reshape(C, NK)
        in_maps.append(m)

    res = run_bass_kernel_spmd(nc, in_maps, core_ids=list(range(NCORES)), trace=_trace)
    out = np.empty((T, B, NQ, C), np.float32)
    for cid in range(NCORES):
        t, b = cid // B, cid % B
        out[t, b] = res.results[cid]["out"].reshape(NQ, C)
    if _trace:
        _CACHE["last_results"] = res
    return out
